# revision 3
# baseline (speedup 1.0000x reference)
"""Trainium2 Bass kernel for EEGToLatentGCN.

Math: the reference stacks all B*C nodes but uses a single 17-node edge_index,
so message passing only touches global nodes 0..16 (batch element 0). Every
other node is a plain per-node MLP:
    h = leaky(x @ We + be); h = leaky(h @ W1 + b1); h = leaky(h @ W2 + b2);
    h = leaky(h @ W3 + b3); g = mean_17(h); out = leaky(g @ Wh1 + bh1) @ Wh2 + bh2
Batch element 0 (17 nodes) is recomputed exactly on the host with the real
graph propagation and overwrites out[0].

v2 layout: x is transposed + bf16-cast on the host to [T, R] so no PE
transpose / DVE copy is needed. All matmuls run bf16 (1 PE cycle/row at any
moving size). The elementwise work (leaky, bias, 17-node pooling) is spread
over ACT + DVE + Pool (GPSIMD):
  ACT : fused bias+LeakyReLU drains of conv1/conv2 PSUM pairs, head g1
  DVE : conv3 pair drain via tensor_tensor(+bias broadcast), embed drain,
        final 9-way tensor_reduce, head output drain with fused +bh2
  Pool: leaky stt second-ops (SBUF only), 8-way pooling pre-reduction
Embed output is "stacked" [128, r/2] (two row-halves on partitions) so its
drain costs half the columns. PSUM: 8 banks exactly (embed 1, conv1/conv3
shared pair 2, conv2 pair 2, head g1 1, head out pair 2).
"""

import numpy as np
import ml_dtypes

import concourse.bass as bass
import concourse.mybir as mybir
import concourse.tile as tile
from concourse import bacc
from concourse.bass_utils import run_bass_kernel_spmd

F32 = mybir.dt.float32
BF16 = mybir.dt.bfloat16
LRELU = mybir.ActivationFunctionType.Lrelu
ADD = mybir.AluOpType.add
MULT = mybir.AluOpType.mult
MAX = mybir.AluOpType.max
AX_X = mybir.AxisListType.X

NCORES = 8
B, C, T, F, H, L = 16384, 17, 80, 64, 256, 1024
BS = B // NCORES      # graphs per core
R = BS * C            # node rows per core
MT_G = 30             # graphs per macro-tile
MT_R = MT_G * C       # 510 rows
N_MT = BS // MT_G     # 68 full macro-tiles
TAIL_G = BS - MT_G * N_MT  # 8
HT_G = 256            # graphs per head tile
SLOPE = 0.01

_CACHE = {}


def _leaky_np(v):
    return np.where(v > 0, v, SLOPE * v)


def _build(reps=1):
    nc = bacc.Bacc("TRN2", target_bir_lowering=False, debug=False)

    x_p = nc.declare_dram_parameter("x", [T, R], BF16, isOutput=False)
    wemb_p = nc.declare_dram_parameter("wemb", [T, F], BF16, isOutput=False)
    bembs_p = nc.declare_dram_parameter("bembs", [128, 1], F32, isOutput=False)
    w1d_p = nc.declare_dram_parameter("w1d", [128, 2, 128], BF16, isOutput=False)
    b1_p = nc.declare_dram_parameter("b1", [128, 2], F32, isOutput=False)
    w2_p = nc.declare_dram_parameter("w2", [128, 2, H], BF16, isOutput=False)
    b2_p = nc.declare_dram_parameter("b2", [128, 2], F32, isOutput=False)
    w3_p = nc.declare_dram_parameter("w3", [128, 2, H], BF16, isOutput=False)
    b3bc_p = nc.declare_dram_parameter("b3bc", [128, 2, MT_R], BF16, isOutput=False)
    wh1_p = nc.declare_dram_parameter("wh1", [128, 2, H], BF16, isOutput=False)
    bh1_p = nc.declare_dram_parameter("bh1", [128, 2], F32, isOutput=False)
    wh2_p = nc.declare_dram_parameter("wh2", [128, 2, L], BF16, isOutput=False)
    bh2bc_p = nc.declare_dram_parameter("bh2bc", [128, 2, 512], BF16, isOutput=False)
    out_p = nc.declare_dram_parameter("out", [BS, L], BF16, isOutput=True)

    with tile.TileContext(nc) as tc:
        with tc.tile_pool(name="consts", bufs=1) as consts:
            wemb_t = consts.tile([T, F], BF16)
            bembs_t = consts.tile([128, 1], F32)
            w1d_t = consts.tile([128, 2, 128], BF16)
            b1_t = consts.tile([128, 2], F32)
            w2_t = consts.tile([128, 2, H], BF16)
            b2_t = consts.tile([128, 2], F32)
            w3_t = consts.tile([128, 2, H], BF16)
            b3bc_t = consts.tile([128, 2, MT_R], BF16)
            wh1_t = consts.tile([128, 2, H], BF16)
            bh1_t = consts.tile([128, 2], F32)
            wh2_t = consts.tile([128, 2, L], BF16)
            bh2bc_t = consts.tile([128, 2, 512], BF16)
            for dst_t, src_p in [
                (wemb_t, wemb_p), (bembs_t, bembs_p), (w1d_t, w1d_p),
                (b1_t, b1_p), (w2_t, w2_p), (b2_t, b2_p),
                (w3_t, w3_p), (b3bc_t, b3bc_p),
            ]:
                nc.sync.dma_start(dst_t[:], src_p[:])

            # pooled per-graph sums (head input), persistent
            gt = consts.tile([128, 2, BS], BF16)

            for _rep in range(reps):
              with tc.tile_pool(name="mwork", bufs=3) as spool, \
                 tc.tile_pool(name="ps_e", bufs=1, space="PSUM") as ps_e_pool, \
                 tc.tile_pool(name="ps_13", bufs=1, space="PSUM") as ps_13_pool, \
                 tc.tile_pool(name="ps_c2", bufs=1, space="PSUM") as ps_c2_pool, \
                 tc.tile_pool(name="hwork", bufs=2) as hpool, \
                 tc.tile_pool(name="ps_g1", bufs=1, space="PSUM") as ps_g1_pool, \
                 tc.tile_pool(name="ps_o", bufs=1, space="PSUM") as ps_o_pool:

                def emit_head(ht):
                    g0 = ht * HT_G
                    ps_g1 = ps_g1_pool.tile([128, 2, HT_G], F32, tag="ps_g1")
                    for c in range(2):
                        nc.tensor.matmul(ps_g1[:, c, :],
                                         wh1_t[:, 0, c * 128:(c + 1) * 128],
                                         gt[:, 0, g0:g0 + HT_G],
                                         start=True, stop=False)
                        nc.tensor.matmul(ps_g1[:, c, :],
                                         wh1_t[:, 1, c * 128:(c + 1) * 128],
                                         gt[:, 1, g0:g0 + HT_G],
                                         start=False, stop=True)
                    g1 = hpool.tile([128, 2, HT_G], BF16, tag="g1")
                    with nc.allow_low_precision(reason="bf16 activations"):
                        for c in range(2):
                            nc.scalar.activation(g1[:, c, :], ps_g1[:, c, :],
                                                 LRELU, bias=bh1_t[:, c:c + 1],
                                                 scale=1.0, alpha=SLOPE)
                    for m in range(HT_G // 128):
                        pso = ps_o_pool.tile([128, 2, 512], F32, tag="ps_o")
                        for nb in range(2):
                            nc.tensor.matmul(
                                pso[:, nb, :],
                                g1[:, 0, m * 128:(m + 1) * 128],
                                wh2_t[:, 0, nb * 512:(nb + 1) * 512],
                                start=True, stop=False)
                            nc.tensor.matmul(
                                pso[:, nb, :],
                                g1[:, 1, m * 128:(m + 1) * 128],
                                wh2_t[:, 1, nb * 512:(nb + 1) * 512],
                                start=False, stop=True)
                        o_sb = hpool.tile([128, 2, 512], BF16, tag="o_sb")
                        with nc.allow_low_precision(reason="bf16 out"):
                            nc.vector.tensor_tensor(
                                o_sb[:], pso[:], bh2bc_t[:], op=ADD)
                        nc.sync.dma_start(
                            out_p[g0 + m * 128:g0 + (m + 1) * 128, :],
                            o_sb[:].rearrange("p a b -> p (a b)"))

                next_ht = 0
                for mt in range(N_MT + (1 if TAIL_G else 0)):
                    g = MT_G if mt < N_MT else TAIL_G
                    g0 = mt * MT_G
                    r0 = g0 * C
                    r = g * C
                    rh = r // 2

                    # x tile [80, 2, rh] bf16 (host pre-transposed)
                    xt = spool.tile([T, 2, MT_R // 2], BF16, tag="xt")
                    nc.sync.dma_start(
                        xt[:, :, 0:rh],
                        x_p[:, r0:r0 + r].rearrange("t (h n) -> t h n", h=2))

                    # embed, stacked: row-halves on partitions 0:64 / 64:128
                    ps_e = ps_e_pool.tile([128, 256], F32, tag="ps_e")
                    for h in range(2):
                        nc.tensor.matmul(ps_e[h * 64:(h + 1) * 64, 0:rh],
                                         wemb_t[:], xt[:, h, 0:rh],
                                         start=True, stop=True)
                    # drain embed: DVE ts(+bias) -> u0, Pool stt leaky -> h0
                    u0 = spool.tile([128, MT_R // 2], BF16, tag="u0")
                    h0 = spool.tile([128, MT_R // 2], BF16, tag="h0")
                    with nc.allow_low_precision(reason="bf16 activations"):
                        nc.vector.tensor_scalar(
                            u0[:, 0:rh], ps_e[:, 0:rh], bembs_t[:, 0:1],
                            None, op0=ADD)
                        nc.gpsimd.scalar_tensor_tensor(
                            h0[:, 0:rh], u0[:, 0:rh], SLOPE, u0[:, 0:rh],
                            op0=MULT, op1=MAX)

                    # conv1 [64]->[256]: 2 chunks x 2 row-halves; ACT drains
                    ps_13 = ps_13_pool.tile([128, 2, 512], F32, tag="ps_13")
                    h1 = spool.tile([128, 2, MT_R], BF16, tag="h1")
                    for c in range(2):
                        for h in range(2):
                            nc.tensor.matmul(
                                ps_13[:, c, h * rh:(h + 1) * rh],
                                w1d_t[h * 64:(h + 1) * 64, c, :],
                                h0[h * 64:(h + 1) * 64, 0:rh],
                                start=True, stop=True)
                        with nc.allow_low_precision(reason="bf16 activations"):
                            nc.scalar.activation(h1[:, c, 0:r],
                                                 ps_13[:, c, 0:r], LRELU,
                                                 bias=b1_t[:, c:c + 1],
                                                 scale=1.0, alpha=SLOPE)

                    # conv2 [256]->[256]; ACT drains
                    ps_c2 = ps_c2_pool.tile([128, 2, 512], F32, tag="ps_c2")
                    h2 = spool.tile([128, 2, MT_R], BF16, tag="h2")
                    for c in range(2):
                        nc.tensor.matmul(ps_c2[:, c, 0:r],
                                         w2_t[:, 0, c * 128:(c + 1) * 128],
                                         h1[:, 0, 0:r], start=True, stop=False)
                        nc.tensor.matmul(ps_c2[:, c, 0:r],
                                         w2_t[:, 1, c * 128:(c + 1) * 128],
                                         h1[:, 1, 0:r], start=False, stop=True)
                        with nc.allow_low_precision(reason="bf16 activations"):
                            nc.scalar.activation(h2[:, c, 0:r],
                                                 ps_c2[:, c, 0:r], LRELU,
                                                 bias=b2_t[:, c:c + 1],
                                                 scale=1.0, alpha=SLOPE)

                    # conv3: pair drain on DVE tt(+b3 broadcast), Pool stt
                    ps_3 = ps_13_pool.tile([128, 2, 512], F32, tag="ps_13")
                    for c in range(2):
                        nc.tensor.matmul(ps_3[:, c, 0:r],
                                         w3_t[:, 0, c * 128:(c + 1) * 128],
                                         h2[:, 0, 0:r], start=True, stop=False)
                        nc.tensor.matmul(ps_3[:, c, 0:r],
                                         w3_t[:, 1, c * 128:(c + 1) * 128],
                                         h2[:, 1, 0:r], start=False, stop=True)
                    u3 = spool.tile([128, 2, MT_R], BF16, tag="u3")
                    h3 = spool.tile([128, 2, MT_R], BF16, tag="h3")
                    with nc.allow_low_precision(reason="bf16 activations"):
                        nc.vector.tensor_tensor(
                            u3[:, :, 0:r], ps_3[:, :, 0:r], b3bc_t[:, :, 0:r],
                            op=ADD)
                        nc.gpsimd.scalar_tensor_tensor(
                            h3[:, :, 0:r], u3[:, :, 0:r], SLOPE, u3[:, :, 0:r],
                            op0=MULT, op1=MAX)

                    # pooling: Pool 8-way pre-add + tail col, DVE 9-reduce
                    h3r = h3[:, :, 0:r].rearrange("p c (g s) -> p c g s", s=C)
                    t9 = spool.tile([128, 2, MT_G, 9], BF16, tag="t9")
                    with nc.allow_low_precision(reason="bf16 pooling"):
                        nc.gpsimd.scalar_tensor_tensor(
                            t9[:, :, 0:g, 0:8], h3r[:, :, :, 0:8], 1.0,
                            h3r[:, :, :, 8:16], op0=MULT, op1=ADD)
                        nc.gpsimd.tensor_copy(
                            t9[:, :, 0:g, 8:9], h3r[:, :, :, 16:17])
                        nc.vector.tensor_reduce(
                            out=gt[:, :, g0:g0 + g],
                            in_=t9[:, :, 0:g, :], op=ADD, axis=AX_X)

                    if _rep == 0 and mt == 1:
                        # head weights deferred so first x tiles get the DMA
                        for dst_t, src_p in [
                            (wh1_t, wh1_p), (bh1_t, bh1_p),
                            (wh2_t, wh2_p), (bh2bc_t, bh2bc_p),
                        ]:
                            nc.sync.dma_start(dst_t[:], src_p[:])

                    done = g0 + g
                    while (next_ht < BS // HT_G
                           and (next_ht + 1) * HT_G <= done):
                        emit_head(next_ht)
                        next_ht += 1

                while next_ht < BS // HT_G:
                    emit_head(next_ht)
                    next_ht += 1

    nc.compile()
    return nc


def _get_nc(reps=1):
    key = ("nc", reps)
    if key not in _CACHE:
        _CACHE[key] = _build(reps)
    return _CACHE[key]


def _fixup_graph0(x, W_emb, b_emb, W1, b1, W2, b2, W3, b3, Wh1, bh1, Wh2, bh2,
                  src, dst):
    """Exact recompute of batch element 0 with real GCN propagation."""
    deg = np.ones(C, np.float64)
    np.add.at(deg, dst.astype(np.int64), 1.0)
    dinv = 1.0 / np.sqrt(deg)
    A = np.zeros((C, C), np.float64)
    A[np.arange(C), np.arange(C)] = dinv * dinv
    np.add.at(A, (dst.astype(np.int64), src.astype(np.int64)),
              dinv[src.astype(np.int64)] * dinv[dst.astype(np.int64)])

    h = _leaky_np(x[0].astype(np.float64) @ W_emb + b_emb)
    for Wc, bc in [(W1, b1), (W2, b2), (W3, b3)]:
        h = _leaky_np(A @ (h @ Wc) + bc)
    g = h.mean(axis=0)
    return (_leaky_np(g @ Wh1 + bh1) @ Wh2 + bh2).astype(np.float32)


def _prep_weights(W_emb, b_emb, W1, b1, W2, b2, W3, b3, Wh1, bh1, Wh2, bh2):
    bf = ml_dtypes.bfloat16

    def kchunks(w):
        # [256, out] -> [128, 2, out] (k-chunk as middle axis)
        return np.ascontiguousarray(
            w.reshape(2, 128, w.shape[1]).transpose(1, 0, 2)).astype(bf)

    def bcols(b):
        # [256] -> [128, 2]
        return np.ascontiguousarray(b.reshape(2, 128).T).astype(np.float32)

    w1d = np.empty((128, 2, 128), np.float32)
    for c in range(2):
        w1d[0:64, c, :] = W1[:, c * 128:(c + 1) * 128]
        w1d[64:128, c, :] = W1[:, c * 128:(c + 1) * 128]

    b3c = bcols(b3)  # [128, 2]
    b3bc = np.ascontiguousarray(
        np.broadcast_to(b3c[:, :, None], (128, 2, MT_R)))

    bh2bc = np.ascontiguousarray(
        np.broadcast_to(bh2.reshape(2, 512)[None, :, :], (128, 2, 512)))

    return {
        "wemb": np.ascontiguousarray(W_emb).astype(bf),
        "bembs": np.ascontiguousarray(
            np.concatenate([b_emb, b_emb]).reshape(128, 1)).astype(np.float32),
        "w1d": np.ascontiguousarray(w1d).astype(bf),
        "b1": bcols(b1),
        "w2": kchunks(W2), "b2": bcols(b2),
        "w3": kchunks(W3), "b3bc": b3bc.astype(bf),
        "wh1": kchunks(Wh1 * (1.0 / C)), "bh1": bcols(bh1),
        "wh2": kchunks(Wh2),
        "bh2bc": bh2bc.astype(bf),
    }


def kernel(x, W_emb, b_emb, W1, b1, W2, b2, W3, b3, Wh1, bh1, Wh2, bh2,
           src, dst):
    bf = ml_dtypes.bfloat16
    x = np.asarray(x, np.float32)
    W_emb = np.asarray(W_emb, np.float32)
    b_emb = np.asarray(b_emb, np.float32)
    W1 = np.asarray(W1, np.float32)
    b1 = np.asarray(b1, np.float32)
    W2 = np.asarray(W2, np.float32)
    b2 = np.asarray(b2, np.float32)
    W3 = np.asarray(W3, np.float32)
    b3 = np.asarray(b3, np.float32)
    Wh1 = np.asarray(Wh1, np.float32)
    bh1 = np.asarray(bh1, np.float32)
    Wh2 = np.asarray(Wh2, np.float32)
    bh2 = np.asarray(bh2, np.float32)

    weights = _prep_weights(W_emb, b_emb, W1, b1, W2, b2, W3, b3,
                            Wh1, bh1, Wh2, bh2)

    # host: [B*C, T] -> [T, B*C] transpose + bf16 cast
    xT = np.ascontiguousarray(x.reshape(B * C, T).T).astype(bf)

    in_maps = []
    for i in range(NCORES):
        m = dict(weights)
        m["x"] = np.ascontiguousarray(xT[:, i * R:(i + 1) * R])
        in_maps.append(m)

    nc = _get_nc()
    res = run_bass_kernel_spmd(nc, in_maps, core_ids=list(range(NCORES)))
    out = np.concatenate(
        [np.asarray(res.results[i]["out"]).astype(np.float32)
         for i in range(NCORES)], axis=0)

    out[0] = _fixup_graph0(x, W_emb, b_emb, W1, b1, W2, b2, W3, b3,
                           Wh1, bh1, Wh2, bh2, np.asarray(src), np.asarray(dst))
    return out


# revision 4
# speedup vs baseline: 1.8890x; 1.8890x over previous
"""Trainium2 Bass kernel for EEGToLatentGCN.

Math: the reference stacks all B*C nodes but uses a single 17-node edge_index,
so message passing only touches global nodes 0..16 (batch element 0). Every
other node is a plain per-node MLP:
    h = leaky(x @ We + be); h = leaky(h @ W1 + b1); h = leaky(h @ W2 + b2);
    h = leaky(h @ W3 + b3); g = mean_17(h); out = leaky(g @ Wh1 + bh1) @ Wh2 + bh2
Batch element 0 (17 nodes) is recomputed exactly on the host with the real
graph propagation and overwrites out[0].

v3: x is host-transposed to [T, R] bf16 (no PE transpose). All matmuls bf16
(1 PE cycle/row at any moving size). Elementwise work is spread over
ACT/DVE/Pool and the emission is SOFTWARE-PIPELINED so each engine works on a
different macro-tile concurrently:
  iter i: [dma x(i+1)] [embed(i) PE; drain DVE ts + Pool stt]
          [pool-finish(i-4): Pool 8-way pre-add + tail copy, DVE 9-reduce]
          [conv2(i-1) PE; ACT drains] [conv3(i-2) PE; DVE tt(+bias) drain;
          Pool stt leaky] [<=1 paced head unit] [conv1(i) PE; ACT drains]
Embed output is stacked [128, r/2] (two row-halves on partitions) to halve its
drain cost. PSUM: 8 banks (embed 1, conv1 2, conv2 2, conv3 2, head 1).
"""

import numpy as np
import ml_dtypes

import concourse.bass as bass
import concourse.mybir as mybir
import concourse.tile as tile
from concourse import bacc
from concourse.bass_utils import run_bass_kernel_spmd

F32 = mybir.dt.float32
BF16 = mybir.dt.bfloat16
LRELU = mybir.ActivationFunctionType.Lrelu
ADD = mybir.AluOpType.add
MULT = mybir.AluOpType.mult
MAX = mybir.AluOpType.max
AX_X = mybir.AxisListType.X

NCORES = 8
B, C, T, F, H, L = 16384, 17, 80, 64, 256, 1024
BS = B // NCORES      # graphs per core
R = BS * C            # node rows per core
MT_G = 30             # graphs per macro-tile
MT_R = MT_G * C       # 510 rows
N_MT = BS // MT_G     # 68 full macro-tiles
TAIL_G = BS - MT_G * N_MT  # 8
NT = N_MT + (1 if TAIL_G else 0)
HT_G = 256            # graphs per head tile
N_HT = BS // HT_G
SLOPE = 0.01

_CACHE = {}


def _leaky_np(v):
    return np.where(v > 0, v, SLOPE * v)


def _mtp(i):
    g = MT_G if i < N_MT else TAIL_G
    g0 = i * MT_G
    return g0, g, g * C, (g * C) // 2


def _build(reps=1):
    nc = bacc.Bacc("TRN2", target_bir_lowering=False, debug=False)

    x_p = nc.declare_dram_parameter("x", [T, R], BF16, isOutput=False)
    wemb_p = nc.declare_dram_parameter("wemb", [T, F], BF16, isOutput=False)
    bembs_p = nc.declare_dram_parameter("bembs", [128, 1], F32, isOutput=False)
    w1d_p = nc.declare_dram_parameter("w1d", [128, 2, 128], BF16, isOutput=False)
    b1_p = nc.declare_dram_parameter("b1", [128, 2], F32, isOutput=False)
    w2_p = nc.declare_dram_parameter("w2", [128, 2, H], BF16, isOutput=False)
    b2_p = nc.declare_dram_parameter("b2", [128, 2], F32, isOutput=False)
    w3_p = nc.declare_dram_parameter("w3", [128, 2, H], BF16, isOutput=False)
    b3bc_p = nc.declare_dram_parameter("b3bc", [128, 2, MT_R], BF16, isOutput=False)
    wh1_p = nc.declare_dram_parameter("wh1", [128, 2, H], BF16, isOutput=False)
    bh1_p = nc.declare_dram_parameter("bh1", [128, 2], F32, isOutput=False)
    wh2_p = nc.declare_dram_parameter("wh2", [128, 2, L], BF16, isOutput=False)
    bh2bc_p = nc.declare_dram_parameter("bh2bc", [128, 2, 512], BF16, isOutput=False)
    out_p = nc.declare_dram_parameter("out", [BS, L], BF16, isOutput=True)

    with tile.TileContext(nc) as tc:
        with tc.tile_pool(name="consts", bufs=1) as consts:
            wemb_t = consts.tile([T, F], BF16)
            bembs_t = consts.tile([128, 1], F32)
            w1d_t = consts.tile([128, 2, 128], BF16)
            b1_t = consts.tile([128, 2], F32)
            w2_t = consts.tile([128, 2, H], BF16)
            b2_t = consts.tile([128, 2], F32)
            w3_t = consts.tile([128, 2, H], BF16)
            b3bc_t = consts.tile([128, 2, MT_R], BF16)
            wh1_t = consts.tile([128, 2, H], BF16)
            bh1_t = consts.tile([128, 2], F32)
            wh2_t = consts.tile([128, 2, L], BF16)
            bh2bc_t = consts.tile([128, 2, 512], BF16)
            for dst_t, src_p in [
                (wemb_t, wemb_p), (bembs_t, bembs_p), (w1d_t, w1d_p),
                (b1_t, b1_p), (w2_t, w2_p), (b2_t, b2_p),
                (w3_t, w3_p), (b3bc_t, b3bc_p),
            ]:
                nc.sync.dma_start(dst_t[:], src_p[:])

            gt = consts.tile([128, 2, BS], BF16)  # pooled sums, persistent

            for _rep in range(reps):
              with tc.tile_pool(name="mwork", bufs=1) as spool, \
                 tc.tile_pool(name="psm", bufs=1, space="PSUM") as psm, \
                 tc.tile_pool(name="hwork", bufs=2) as hpool:

                st = {}          # per-mt live tiles
                hstate = {}      # head tiles

                def dma_x(i):
                    g0, g, r, rh = _mtp(i)
                    xt = spool.tile([T, 2, MT_R // 2], BF16, tag="xt", bufs=3)
                    nc.sync.dma_start(
                        xt[:, :, 0:rh],
                        x_p[:, g0 * C:g0 * C + r].rearrange(
                            "t (h n) -> t h n", h=2))
                    st[("xt", i)] = xt

                def front_a(i):
                    g0, g, r, rh = _mtp(i)
                    xt = st.pop(("xt", i))
                    ps_e = psm.tile([128, 512], F32, tag="e", bufs=1)
                    for h in range(2):
                        nc.tensor.matmul(ps_e[h * 64:(h + 1) * 64, 0:rh],
                                         wemb_t[:], xt[:, h, 0:rh],
                                         start=True, stop=True)
                    u0 = spool.tile([128, MT_R // 2], BF16, tag="u0", bufs=2)
                    h0 = spool.tile([128, MT_R // 2], BF16, tag="h0", bufs=2)
                    with nc.allow_low_precision(reason="bf16 activations"):
                        nc.vector.tensor_scalar(
                            u0[:, 0:rh], ps_e[:, 0:rh], bembs_t[:, 0:1],
                            None, op0=ADD)
                        nc.gpsimd.scalar_tensor_tensor(
                            h0[:, 0:rh], u0[:, 0:rh], SLOPE, u0[:, 0:rh],
                            op0=MULT, op1=MAX)
                    st[("h0", i)] = h0

                def front_b(i):
                    g0, g, r, rh = _mtp(i)
                    h0 = st.pop(("h0", i))
                    ps_c1 = psm.tile([128, 2, 512], F32, tag="c1", bufs=1)
                    h1 = spool.tile([128, 2, MT_R], BF16, tag="h1", bufs=2)
                    for c in range(2):
                        for h in range(2):
                            nc.tensor.matmul(
                                ps_c1[:, c, h * rh:(h + 1) * rh],
                                w1d_t[h * 64:(h + 1) * 64, c, :],
                                h0[h * 64:(h + 1) * 64, 0:rh],
                                start=True, stop=True)
                        with nc.allow_low_precision(reason="bf16 activations"):
                            nc.scalar.activation(h1[:, c, 0:r],
                                                 ps_c1[:, c, 0:r], LRELU,
                                                 bias=b1_t[:, c:c + 1],
                                                 scale=1.0, alpha=SLOPE)
                    st[("h1", i)] = h1

                def mid(i):
                    g0, g, r, rh = _mtp(i)
                    h1 = st.pop(("h1", i))
                    ps_c2 = psm.tile([128, 2, 512], F32, tag="c2", bufs=1)
                    h2 = spool.tile([128, 2, MT_R], BF16, tag="h2", bufs=2)
                    for c in range(2):
                        nc.tensor.matmul(ps_c2[:, c, 0:r],
                                         w2_t[:, 0, c * 128:(c + 1) * 128],
                                         h1[:, 0, 0:r], start=True, stop=False)
                        nc.tensor.matmul(ps_c2[:, c, 0:r],
                                         w2_t[:, 1, c * 128:(c + 1) * 128],
                                         h1[:, 1, 0:r], start=False, stop=True)
                        with nc.allow_low_precision(reason="bf16 activations"):
                            nc.scalar.activation(h2[:, c, 0:r],
                                                 ps_c2[:, c, 0:r], LRELU,
                                                 bias=b2_t[:, c:c + 1],
                                                 scale=1.0, alpha=SLOPE)
                    st[("h2", i)] = h2

                def back(i):
                    g0, g, r, rh = _mtp(i)
                    h2 = st.pop(("h2", i))
                    ps_c3 = psm.tile([128, 2, 512], F32, tag="c3", bufs=1)
                    for c in range(2):
                        nc.tensor.matmul(ps_c3[:, c, 0:r],
                                         w3_t[:, 0, c * 128:(c + 1) * 128],
                                         h2[:, 0, 0:r], start=True, stop=False)
                        nc.tensor.matmul(ps_c3[:, c, 0:r],
                                         w3_t[:, 1, c * 128:(c + 1) * 128],
                                         h2[:, 1, 0:r], start=False, stop=True)
                    u3 = spool.tile([128, 2, MT_R], BF16, tag="u3", bufs=2)
                    h3 = spool.tile([128, 2, MT_R], BF16, tag="h3", bufs=3)
                    with nc.allow_low_precision(reason="bf16 activations"):
                        nc.vector.tensor_tensor(
                            u3[:, :, 0:r], ps_c3[:, :, 0:r], b3bc_t[:, :, 0:r],
                            op=ADD)
                        nc.gpsimd.scalar_tensor_tensor(
                            h3[:, :, 0:r], u3[:, :, 0:r], SLOPE, u3[:, :, 0:r],
                            op0=MULT, op1=MAX)
                    st[("h3", i)] = h3

                def pool_finish(i):
                    g0, g, r, rh = _mtp(i)
                    h3 = st.pop(("h3", i))
                    h3r = h3[:, :, 0:r].rearrange("p c (g s) -> p c g s", s=C)
                    t9 = spool.tile([128, 2, MT_G, 9], BF16, tag="t9", bufs=2)
                    with nc.allow_low_precision(reason="bf16 pooling"):
                        nc.gpsimd.scalar_tensor_tensor(
                            t9[:, :, 0:g, 0:8], h3r[:, :, :, 0:8], 1.0,
                            h3r[:, :, :, 8:16], op0=MULT, op1=ADD)
                        nc.gpsimd.tensor_copy(
                            t9[:, :, 0:g, 8:9], h3r[:, :, :, 16:17])
                        nc.vector.tensor_reduce(
                            out=gt[:, :, g0:g0 + g],
                            in_=t9[:, :, 0:g, :], op=ADD, axis=AX_X)

                # ---- head: paced units ----
                units = []
                for ht in range(N_HT):
                    units.append(("g1", ht, 0, 0))
                    for m in range(HT_G // 128):
                        for nb in range(2):
                            units.append(("o", ht, m, nb))

                def unit_ready(k, reduced_mts):
                    if k >= len(units):
                        return False
                    ht = units[k][1]
                    need_g = (ht + 1) * HT_G
                    m_req = -(-need_g // MT_G)  # ceil
                    return reduced_mts >= min(m_req + 2, NT)

                def emit_unit(k):
                    kind, ht, m, nb = units[k]
                    g0 = ht * HT_G
                    if kind == "g1":
                        ps_g1 = psm.tile([128, 512], F32, tag="hd", bufs=1)
                        for c in range(2):
                            nc.tensor.matmul(
                                ps_g1[:, c * 256:c * 256 + 256],
                                wh1_t[:, 0, c * 128:(c + 1) * 128],
                                gt[:, 0, g0:g0 + HT_G],
                                start=True, stop=False)
                            nc.tensor.matmul(
                                ps_g1[:, c * 256:c * 256 + 256],
                                wh1_t[:, 1, c * 128:(c + 1) * 128],
                                gt[:, 1, g0:g0 + HT_G],
                                start=False, stop=True)
                        g1 = hpool.tile([128, 2, HT_G], BF16, tag="g1")
                        with nc.allow_low_precision(reason="bf16"):
                            for c in range(2):
                                nc.scalar.activation(
                                    g1[:, c, :], ps_g1[:, c * 256:c * 256 + 256],
                                    LRELU, bias=bh1_t[:, c:c + 1],
                                    scale=1.0, alpha=SLOPE)
                        hstate[("g1", ht)] = g1
                    else:
                        g1 = hstate[("g1", ht)]
                        pso = psm.tile([128, 512], F32, tag="hd", bufs=1)
                        nc.tensor.matmul(
                            pso[:],
                            g1[:, 0, m * 128:(m + 1) * 128],
                            wh2_t[:, 0, nb * 512:(nb + 1) * 512],
                            start=True, stop=False)
                        nc.tensor.matmul(
                            pso[:],
                            g1[:, 1, m * 128:(m + 1) * 128],
                            wh2_t[:, 1, nb * 512:(nb + 1) * 512],
                            start=False, stop=True)
                        if nb == 0:
                            o_sb = hpool.tile([128, 2, 512], BF16, tag="o_sb")
                            hstate[("o", ht, m)] = o_sb
                        else:
                            o_sb = hstate.pop(("o", ht, m))
                        with nc.allow_low_precision(reason="bf16 out"):
                            nc.vector.tensor_tensor(
                                o_sb[:, nb, :], pso[:], bh2bc_t[:, nb, :],
                                op=ADD)
                        if nb == 1:
                            nc.sync.dma_start(
                                out_p[g0 + m * 128:g0 + (m + 1) * 128, :],
                                o_sb[:].rearrange("p a b -> p (a b)"))

                # ---- pipelined emission ----
                dma_x(0)
                next_unit = 0
                for i in range(NT + 5):
                    if i + 1 < NT:
                        dma_x(i + 1)
                    if i < NT:
                        front_a(i)
                    if 0 <= i - 4 and i - 4 < NT:
                        pool_finish(i - 4)
                    reduced = min(i - 3, NT)  # pool_finish emitted count
                    if 0 <= i - 1 and i - 1 < NT:
                        mid(i - 1)
                    if 0 <= i - 2 and i - 2 < NT:
                        back(i - 2)
                    budget = 1 if i < NT else 4
                    while budget > 0 and unit_ready(next_unit, reduced):
                        emit_unit(next_unit)
                        next_unit += 1
                        budget -= 1
                    if _rep == 0 and i == 2:
                        for dst_t, src_p in [
                            (wh1_t, wh1_p), (bh1_t, bh1_p),
                            (wh2_t, wh2_p), (bh2bc_t, bh2bc_p),
                        ]:
                            nc.sync.dma_start(dst_t[:], src_p[:])
                    if i < NT:
                        front_b(i)
                while next_unit < len(units):
                    emit_unit(next_unit)
                    next_unit += 1

    nc.compile()
    return nc


def _get_nc(reps=1):
    key = ("nc", reps)
    if key not in _CACHE:
        _CACHE[key] = _build(reps)
    return _CACHE[key]


def _fixup_graph0(x, W_emb, b_emb, W1, b1, W2, b2, W3, b3, Wh1, bh1, Wh2, bh2,
                  src, dst):
    """Exact recompute of batch element 0 with real GCN propagation."""
    deg = np.ones(C, np.float64)
    np.add.at(deg, dst.astype(np.int64), 1.0)
    dinv = 1.0 / np.sqrt(deg)
    A = np.zeros((C, C), np.float64)
    A[np.arange(C), np.arange(C)] = dinv * dinv
    np.add.at(A, (dst.astype(np.int64), src.astype(np.int64)),
              dinv[src.astype(np.int64)] * dinv[dst.astype(np.int64)])

    h = _leaky_np(x[0].astype(np.float64) @ W_emb + b_emb)
    for Wc, bc in [(W1, b1), (W2, b2), (W3, b3)]:
        h = _leaky_np(A @ (h @ Wc) + bc)
    g = h.mean(axis=0)
    return (_leaky_np(g @ Wh1 + bh1) @ Wh2 + bh2).astype(np.float32)


def _prep_weights(W_emb, b_emb, W1, b1, W2, b2, W3, b3, Wh1, bh1, Wh2, bh2):
    bf = ml_dtypes.bfloat16

    def kchunks(w):
        # [256, out] -> [128, 2, out] (k-chunk as middle axis)
        return np.ascontiguousarray(
            w.reshape(2, 128, w.shape[1]).transpose(1, 0, 2)).astype(bf)

    def bcols(b):
        # [256] -> [128, 2]
        return np.ascontiguousarray(b.reshape(2, 128).T).astype(np.float32)

    w1d = np.empty((128, 2, 128), np.float32)
    for c in range(2):
        w1d[0:64, c, :] = W1[:, c * 128:(c + 1) * 128]
        w1d[64:128, c, :] = W1[:, c * 128:(c + 1) * 128]

    b3c = bcols(b3)  # [128, 2]
    b3bc = np.ascontiguousarray(
        np.broadcast_to(b3c[:, :, None], (128, 2, MT_R)))

    bh2bc = np.ascontiguousarray(
        np.broadcast_to(bh2.reshape(2, 512)[None, :, :], (128, 2, 512)))

    return {
        "wemb": np.ascontiguousarray(W_emb).astype(bf),
        "bembs": np.ascontiguousarray(
            np.concatenate([b_emb, b_emb]).reshape(128, 1)).astype(np.float32),
        "w1d": np.ascontiguousarray(w1d).astype(bf),
        "b1": bcols(b1),
        "w2": kchunks(W2), "b2": bcols(b2),
        "w3": kchunks(W3), "b3bc": b3bc.astype(bf),
        "wh1": kchunks(Wh1 * (1.0 / C)), "bh1": bcols(bh1),
        "wh2": kchunks(Wh2),
        "bh2bc": bh2bc.astype(bf),
    }


def kernel(x, W_emb, b_emb, W1, b1, W2, b2, W3, b3, Wh1, bh1, Wh2, bh2,
           src, dst):
    bf = ml_dtypes.bfloat16
    x = np.asarray(x, np.float32)
    W_emb = np.asarray(W_emb, np.float32)
    b_emb = np.asarray(b_emb, np.float32)
    W1 = np.asarray(W1, np.float32)
    b1 = np.asarray(b1, np.float32)
    W2 = np.asarray(W2, np.float32)
    b2 = np.asarray(b2, np.float32)
    W3 = np.asarray(W3, np.float32)
    b3 = np.asarray(b3, np.float32)
    Wh1 = np.asarray(Wh1, np.float32)
    bh1 = np.asarray(bh1, np.float32)
    Wh2 = np.asarray(Wh2, np.float32)
    bh2 = np.asarray(bh2, np.float32)

    weights = _prep_weights(W_emb, b_emb, W1, b1, W2, b2, W3, b3,
                            Wh1, bh1, Wh2, bh2)

    # host: [B*C, T] -> [T, B*C] transpose + bf16 cast
    xT = np.ascontiguousarray(x.reshape(B * C, T).T).astype(bf)

    in_maps = []
    for i in range(NCORES):
        m = dict(weights)
        m["x"] = np.ascontiguousarray(xT[:, i * R:(i + 1) * R])
        in_maps.append(m)

    nc = _get_nc()
    res = run_bass_kernel_spmd(nc, in_maps, core_ids=list(range(NCORES)))
    out = np.concatenate(
        [np.asarray(res.results[i]["out"]).astype(np.float32)
         for i in range(NCORES)], axis=0)

    out[0] = _fixup_graph0(x, W_emb, b_emb, W1, b1, W2, b2, W3, b3,
                           Wh1, bh1, Wh2, bh2, np.asarray(src), np.asarray(dst))
    return out


# revision 8
# speedup vs baseline: 2.0104x; 1.0643x over previous
"""Trainium2 Bass kernel for EEGToLatentGCN.

Math: the reference stacks all B*C nodes but uses a single 17-node edge_index,
so message passing only touches global nodes 0..16 (batch element 0). Every
other node is a plain per-node MLP:
    h = leaky(x @ We + be); h = leaky(h @ W1 + b1); h = leaky(h @ W2 + b2);
    h = leaky(h @ W3 + b3); g = mean_17(h); out = leaky(g @ Wh1 + bh1) @ Wh2 + bh2
Batch element 0 (17 nodes) is recomputed exactly on the host with the real
graph propagation and overwrites out[0].

v3: x is host-transposed to [T, R] bf16 (no PE transpose). All matmuls bf16
(1 PE cycle/row at any moving size). Elementwise work is spread over
ACT/DVE/Pool and the emission is SOFTWARE-PIPELINED so each engine works on a
different macro-tile concurrently:
  iter i: [dma x(i+1)] [embed(i) PE; drain DVE ts + Pool stt]
          [pool-finish(i-4): Pool 8-way pre-add + tail copy, DVE 9-reduce]
          [conv2(i-1) PE; ACT drains] [conv3(i-2) PE; DVE tt(+bias) drain;
          Pool stt leaky] [<=1 paced head unit] [conv1(i) PE; ACT drains]
Embed output is stacked [128, r/2] (two row-halves on partitions) to halve its
drain cost. PSUM: 8 banks (embed 1, conv1 2, conv2 2, conv3 2, head 1).
"""

import numpy as np
import ml_dtypes

import concourse.bass as bass
import concourse.mybir as mybir
import concourse.tile as tile
from concourse import bacc
from concourse.bass_utils import run_bass_kernel_spmd

F32 = mybir.dt.float32
BF16 = mybir.dt.bfloat16
LRELU = mybir.ActivationFunctionType.Lrelu
ADD = mybir.AluOpType.add
MULT = mybir.AluOpType.mult
MAX = mybir.AluOpType.max
AX_X = mybir.AxisListType.X

NCORES = 8
B, C, T, F, H, L = 16384, 17, 80, 64, 256, 1024
BS = B // NCORES      # graphs per core
R = BS * C            # node rows per core
MT_G = 30             # graphs per macro-tile
MT_R = MT_G * C       # 510 rows
N_MT = BS // MT_G     # 68 full macro-tiles
TAIL_G = BS - MT_G * N_MT  # 8
NT = N_MT + (1 if TAIL_G else 0)
HT_G = 256            # graphs per head tile
N_HT = BS // HT_G
SLOPE = 0.01

_CACHE = {}


def _leaky_np(v):
    return np.where(v > 0, v, SLOPE * v)


def _mtp(i):
    g = MT_G if i < N_MT else TAIL_G
    g0 = i * MT_G
    return g0, g, g * C, (g * C) // 2


def _build(reps=1):
    nc = bacc.Bacc("TRN2", target_bir_lowering=False, debug=False)

    x_p = nc.declare_dram_parameter("x", [T, R], BF16, isOutput=False)
    wemb_p = nc.declare_dram_parameter("wemb", [T, F], BF16, isOutput=False)
    bembs_p = nc.declare_dram_parameter("bembs", [128, 1], F32, isOutput=False)
    w1d_p = nc.declare_dram_parameter("w1d", [128, 2, 128], BF16, isOutput=False)
    b1_p = nc.declare_dram_parameter("b1", [128, 2], F32, isOutput=False)
    w2_p = nc.declare_dram_parameter("w2", [128, 2, H], BF16, isOutput=False)
    b2_p = nc.declare_dram_parameter("b2", [128, 2], F32, isOutput=False)
    w3_p = nc.declare_dram_parameter("w3", [128, 2, H], BF16, isOutput=False)
    b3bc_p = nc.declare_dram_parameter("b3bc", [128, 2, MT_R], BF16, isOutput=False)
    wh1_p = nc.declare_dram_parameter("wh1", [128, 2, H], BF16, isOutput=False)
    bh1_p = nc.declare_dram_parameter("bh1", [128, 2], F32, isOutput=False)
    wh2_p = nc.declare_dram_parameter("wh2", [128, 2, L], BF16, isOutput=False)
    bh2bc_p = nc.declare_dram_parameter("bh2bc", [128, 2, 512], BF16, isOutput=False)
    out_p = nc.declare_dram_parameter("out", [BS, L], BF16, isOutput=True)

    with tile.TileContext(nc) as tc:
        with tc.tile_pool(name="consts", bufs=1) as consts:
            wemb_t = consts.tile([T, F], BF16)
            bembs_t = consts.tile([128, 1], F32)
            w1d_t = consts.tile([128, 2, 128], BF16)
            b1_t = consts.tile([128, 2], F32)
            w2_t = consts.tile([128, 2, H], BF16)
            b2_t = consts.tile([128, 2], F32)
            w3_t = consts.tile([128, 2, H], BF16)
            b3bc_t = consts.tile([128, 2, MT_R], BF16)
            wh1_t = consts.tile([128, 2, H], BF16)
            bh1_t = consts.tile([128, 2], F32)
            wh2_t = consts.tile([128, 2, L], BF16)
            bh2bc_t = consts.tile([128, 2, 512], BF16)
            for dst_t, src_p in [
                (wemb_t, wemb_p), (bembs_t, bembs_p), (w1d_t, w1d_p),
                (b1_t, b1_p), (w2_t, w2_p), (b2_t, b2_p),
                (w3_t, w3_p), (b3bc_t, b3bc_p),
            ]:
                nc.sync.dma_start(dst_t[:], src_p[:])

            gt = consts.tile([128, 2, BS], BF16)  # pooled sums, persistent

            for _rep in range(reps):
              with tc.tile_pool(name="mwork", bufs=1) as spool, \
                 tc.tile_pool(name="psm", bufs=1, space="PSUM") as psm, \
                 tc.tile_pool(name="hwork", bufs=2) as hpool:

                st = {}          # per-mt live tiles
                hstate = {}      # head tiles

                def dma_x(i):
                    g0, g, r, rh = _mtp(i)
                    xt = spool.tile([T, 2, MT_R // 2], BF16, tag="xt", bufs=3)
                    nc.sync.dma_start(
                        xt[:, :, 0:rh],
                        x_p[:, g0 * C:g0 * C + r].rearrange(
                            "t (h n) -> t h n", h=2))
                    st[("xt", i)] = xt

                def front_a(i):
                    g0, g, r, rh = _mtp(i)
                    xt = st.pop(("xt", i))
                    ps_e = psm.tile([128, 512], F32, tag="e", bufs=1)
                    for h in range(2):
                        nc.tensor.matmul(ps_e[h * 64:(h + 1) * 64, 0:rh],
                                         wemb_t[:], xt[:, h, 0:rh],
                                         start=True, stop=True)
                    u0 = spool.tile([128, MT_R // 2], BF16, tag="u0", bufs=2)
                    h0 = spool.tile([128, MT_R // 2], BF16, tag="h0", bufs=2)
                    with nc.allow_low_precision(reason="bf16 activations"):
                        nc.vector.tensor_scalar(
                            u0[:, 0:rh], ps_e[:, 0:rh], bembs_t[:, 0:1],
                            None, op0=ADD)
                        nc.gpsimd.scalar_tensor_tensor(
                            h0[:, 0:rh], u0[:, 0:rh], SLOPE, u0[:, 0:rh],
                            op0=MULT, op1=MAX)
                    st[("h0", i)] = h0

                def front_b(i):
                    g0, g, r, rh = _mtp(i)
                    h0 = st.pop(("h0", i))
                    ps_c1 = psm.tile([128, 2, 512], F32, tag="c1", bufs=1)
                    h1 = spool.tile([128, 2, MT_R], BF16, tag="h1", bufs=2)
                    for c in range(2):
                        for h in range(2):
                            nc.tensor.matmul(
                                ps_c1[:, c, h * rh:(h + 1) * rh],
                                w1d_t[h * 64:(h + 1) * 64, c, :],
                                h0[h * 64:(h + 1) * 64, 0:rh],
                                start=True, stop=True)
                        with nc.allow_low_precision(reason="bf16 activations"):
                            nc.scalar.activation(h1[:, c, 0:r],
                                                 ps_c1[:, c, 0:r], LRELU,
                                                 bias=b1_t[:, c:c + 1],
                                                 scale=1.0, alpha=SLOPE)
                    st[("h1", i)] = h1

                def mid(i):
                    g0, g, r, rh = _mtp(i)
                    h1 = st.pop(("h1", i))
                    ps_c2 = psm.tile([128, 2, 512], F32, tag="c2", bufs=1)
                    h2 = spool.tile([128, 2, MT_R], BF16, tag="h2", bufs=2)
                    for c in range(2):
                        nc.tensor.matmul(ps_c2[:, c, 0:r],
                                         w2_t[:, 0, c * 128:(c + 1) * 128],
                                         h1[:, 0, 0:r], start=True, stop=False)
                        nc.tensor.matmul(ps_c2[:, c, 0:r],
                                         w2_t[:, 1, c * 128:(c + 1) * 128],
                                         h1[:, 1, 0:r], start=False, stop=True)
                        with nc.allow_low_precision(reason="bf16 activations"):
                            nc.scalar.activation(h2[:, c, 0:r],
                                                 ps_c2[:, c, 0:r], LRELU,
                                                 bias=b2_t[:, c:c + 1],
                                                 scale=1.0, alpha=SLOPE)
                    st[("h2", i)] = h2

                def back(i):
                    g0, g, r, rh = _mtp(i)
                    h2 = st.pop(("h2", i))
                    ps_c3 = psm.tile([128, 2, 512], F32, tag="c3", bufs=1)
                    for c in range(2):
                        nc.tensor.matmul(ps_c3[:, c, 0:r],
                                         w3_t[:, 0, c * 128:(c + 1) * 128],
                                         h2[:, 0, 0:r], start=True, stop=False)
                        nc.tensor.matmul(ps_c3[:, c, 0:r],
                                         w3_t[:, 1, c * 128:(c + 1) * 128],
                                         h2[:, 1, 0:r], start=False, stop=True)
                    u3 = spool.tile([128, 2, MT_R], BF16, tag="u3", bufs=2)
                    h3 = spool.tile([128, 2, MT_R], BF16, tag="h3", bufs=3)
                    with nc.allow_low_precision(reason="bf16 activations"):
                        nc.vector.tensor_tensor(
                            u3[:, :, 0:r], ps_c3[:, :, 0:r], b3bc_t[:, :, 0:r],
                            op=ADD)
                        nc.gpsimd.scalar_tensor_tensor(
                            h3[:, :, 0:r], u3[:, :, 0:r], SLOPE, u3[:, :, 0:r],
                            op0=MULT, op1=MAX)
                    st[("h3", i)] = h3

                def pool_finish(i):
                    g0, g, r, rh = _mtp(i)
                    h3 = st.pop(("h3", i))
                    h3r = h3[:, :, 0:r].rearrange("p c (g s) -> p c g s", s=C)
                    t9 = spool.tile([128, 2, MT_G, 9], BF16, tag="t9", bufs=2)
                    with nc.allow_low_precision(reason="bf16 pooling"):
                        nc.gpsimd.tensor_copy(
                            t9[:, :, 0:g, 8:9], h3r[:, :, :, 16:17])
                        nc.vector.tensor_tensor(
                            t9[:, :, 0:g, 0:8], h3r[:, :, :, 0:8],
                            h3r[:, :, :, 8:16], op=ADD)
                        nc.vector.tensor_reduce(
                            out=gt[:, :, g0:g0 + g],
                            in_=t9[:, :, 0:g, :], op=ADD, axis=AX_X)

                # ---- head: paced units ----
                units = []
                for ht in range(N_HT):
                    units.append(("g1", ht, 0, 0))
                    for m in range(HT_G // 128):
                        for nb in range(2):
                            units.append(("o", ht, m, nb))

                def unit_ready(k, reduced_mts):
                    if k >= len(units):
                        return False
                    ht = units[k][1]
                    need_g = (ht + 1) * HT_G
                    m_req = -(-need_g // MT_G)  # ceil
                    return reduced_mts >= min(m_req + 2, NT)

                def emit_unit(k, alt_bank=False):
                    kind, ht, m, nb = units[k]
                    g0 = ht * HT_G
                    if kind == "g1":
                        ps_g1 = psm.tile([128, 512], F32, tag="hd", bufs=1)
                        for c in range(2):
                            nc.tensor.matmul(
                                ps_g1[:, c * 256:c * 256 + 256],
                                wh1_t[:, 0, c * 128:(c + 1) * 128],
                                gt[:, 0, g0:g0 + HT_G],
                                start=True, stop=False)
                            nc.tensor.matmul(
                                ps_g1[:, c * 256:c * 256 + 256],
                                wh1_t[:, 1, c * 128:(c + 1) * 128],
                                gt[:, 1, g0:g0 + HT_G],
                                start=False, stop=True)
                        g1 = hpool.tile([128, 2, HT_G], BF16, tag="g1")
                        with nc.allow_low_precision(reason="bf16"):
                            for c in range(2):
                                nc.scalar.activation(
                                    g1[:, c, :], ps_g1[:, c * 256:c * 256 + 256],
                                    LRELU, bias=bh1_t[:, c:c + 1],
                                    scale=1.0, alpha=SLOPE)
                        hstate[("g1", ht)] = g1
                    else:
                        g1 = hstate[("g1", ht)]
                        if alt_bank:
                            # epilogue: conv2 banks are idle; alternate so
                            # consecutive o-units overlap mm with drain
                            ps2 = psm.tile([128, 2, 512], F32, tag="c2", bufs=1)
                            pso = ps2[:, 0, :]
                        else:
                            pso = psm.tile([128, 512], F32, tag="hd", bufs=1)
                        nc.tensor.matmul(
                            pso[:],
                            g1[:, 0, m * 128:(m + 1) * 128],
                            wh2_t[:, 0, nb * 512:(nb + 1) * 512],
                            start=True, stop=False)
                        nc.tensor.matmul(
                            pso[:],
                            g1[:, 1, m * 128:(m + 1) * 128],
                            wh2_t[:, 1, nb * 512:(nb + 1) * 512],
                            start=False, stop=True)
                        if nb == 0:
                            o_sb = hpool.tile([128, 2, 512], BF16, tag="o_sb")
                            hstate[("o", ht, m)] = o_sb
                        else:
                            o_sb = hstate.pop(("o", ht, m))
                        with nc.allow_low_precision(reason="bf16 out"):
                            nc.vector.tensor_tensor(
                                o_sb[:, nb, :], pso[:], bh2bc_t[:, nb, :],
                                op=ADD)
                        if nb == 1:
                            nc.sync.dma_start(
                                out_p[g0 + m * 128:g0 + (m + 1) * 128, :],
                                o_sb[:].rearrange("p a b -> p (a b)"))

                # ---- pipelined emission ----
                dma_x(0)
                next_unit = 0
                for i in range(NT + 5):
                    if i + 1 < NT:
                        dma_x(i + 1)
                    if i < NT:
                        front_a(i)
                    if 0 <= i - 1 and i - 1 < NT:
                        mid(i - 1)
                    if 0 <= i - 2 and i - 2 < NT:
                        back(i - 2)
                    if 0 <= i - 4 and i - 4 < NT:
                        pool_finish(i - 4)
                    reduced = min(i - 3, NT)  # pool_finish emitted count
                    budget = 1 if i < NT else 4
                    while budget > 0 and unit_ready(next_unit, reduced):
                        emit_unit(next_unit, alt_bank=(i >= NT
                                                      and next_unit % 2 == 1))
                        next_unit += 1
                        budget -= 1
                    if _rep == 0 and i == 2:
                        for dst_t, src_p in [
                            (wh1_t, wh1_p), (bh1_t, bh1_p),
                            (wh2_t, wh2_p), (bh2bc_t, bh2bc_p),
                        ]:
                            nc.sync.dma_start(dst_t[:], src_p[:])
                    if i < NT:
                        front_b(i)
                while next_unit < len(units):
                    emit_unit(next_unit, alt_bank=(next_unit % 2 == 1))
                    next_unit += 1

    nc.compile()
    return nc


def _get_nc(reps=1):
    key = ("nc", reps)
    if key not in _CACHE:
        _CACHE[key] = _build(reps)
    return _CACHE[key]


def _fixup_graph0(x, W_emb, b_emb, W1, b1, W2, b2, W3, b3, Wh1, bh1, Wh2, bh2,
                  src, dst):
    """Exact recompute of batch element 0 with real GCN propagation."""
    deg = np.ones(C, np.float64)
    np.add.at(deg, dst.astype(np.int64), 1.0)
    dinv = 1.0 / np.sqrt(deg)
    A = np.zeros((C, C), np.float64)
    A[np.arange(C), np.arange(C)] = dinv * dinv
    np.add.at(A, (dst.astype(np.int64), src.astype(np.int64)),
              dinv[src.astype(np.int64)] * dinv[dst.astype(np.int64)])

    h = _leaky_np(x[0].astype(np.float64) @ W_emb + b_emb)
    for Wc, bc in [(W1, b1), (W2, b2), (W3, b3)]:
        h = _leaky_np(A @ (h @ Wc) + bc)
    g = h.mean(axis=0)
    return (_leaky_np(g @ Wh1 + bh1) @ Wh2 + bh2).astype(np.float32)


def _prep_weights(W_emb, b_emb, W1, b1, W2, b2, W3, b3, Wh1, bh1, Wh2, bh2):
    bf = ml_dtypes.bfloat16

    def kchunks(w):
        # [256, out] -> [128, 2, out] (k-chunk as middle axis)
        return np.ascontiguousarray(
            w.reshape(2, 128, w.shape[1]).transpose(1, 0, 2)).astype(bf)

    def bcols(b):
        # [256] -> [128, 2]
        return np.ascontiguousarray(b.reshape(2, 128).T).astype(np.float32)

    w1d = np.empty((128, 2, 128), np.float32)
    for c in range(2):
        w1d[0:64, c, :] = W1[:, c * 128:(c + 1) * 128]
        w1d[64:128, c, :] = W1[:, c * 128:(c + 1) * 128]

    b3c = bcols(b3)  # [128, 2]
    b3bc = np.ascontiguousarray(
        np.broadcast_to(b3c[:, :, None], (128, 2, MT_R)))

    bh2bc = np.ascontiguousarray(
        np.broadcast_to(bh2.reshape(2, 512)[None, :, :], (128, 2, 512)))

    return {
        "wemb": np.ascontiguousarray(W_emb).astype(bf),
        "bembs": np.ascontiguousarray(
            np.concatenate([b_emb, b_emb]).reshape(128, 1)).astype(np.float32),
        "w1d": np.ascontiguousarray(w1d).astype(bf),
        "b1": bcols(b1),
        "w2": kchunks(W2), "b2": bcols(b2),
        "w3": kchunks(W3), "b3bc": b3bc.astype(bf),
        "wh1": kchunks(Wh1 * (1.0 / C)), "bh1": bcols(bh1),
        "wh2": kchunks(Wh2),
        "bh2bc": bh2bc.astype(bf),
    }


def kernel(x, W_emb, b_emb, W1, b1, W2, b2, W3, b3, Wh1, bh1, Wh2, bh2,
           src, dst):
    bf = ml_dtypes.bfloat16
    x = np.asarray(x, np.float32)
    W_emb = np.asarray(W_emb, np.float32)
    b_emb = np.asarray(b_emb, np.float32)
    W1 = np.asarray(W1, np.float32)
    b1 = np.asarray(b1, np.float32)
    W2 = np.asarray(W2, np.float32)
    b2 = np.asarray(b2, np.float32)
    W3 = np.asarray(W3, np.float32)
    b3 = np.asarray(b3, np.float32)
    Wh1 = np.asarray(Wh1, np.float32)
    bh1 = np.asarray(bh1, np.float32)
    Wh2 = np.asarray(Wh2, np.float32)
    bh2 = np.asarray(bh2, np.float32)

    weights = _prep_weights(W_emb, b_emb, W1, b1, W2, b2, W3, b3,
                            Wh1, bh1, Wh2, bh2)

    # host: [B*C, T] -> [T, B*C] transpose + bf16 cast
    xT = np.ascontiguousarray(x.reshape(B * C, T).T).astype(bf)

    in_maps = []
    for i in range(NCORES):
        m = dict(weights)
        m["x"] = np.ascontiguousarray(xT[:, i * R:(i + 1) * R])
        in_maps.append(m)

    nc = _get_nc()
    res = run_bass_kernel_spmd(nc, in_maps, core_ids=list(range(NCORES)))
    out = np.concatenate(
        [np.asarray(res.results[i]["out"]).astype(np.float32)
         for i in range(NCORES)], axis=0)

    out[0] = _fixup_graph0(x, W_emb, b_emb, W1, b1, W2, b2, W3, b3,
                           Wh1, bh1, Wh2, bh2, np.asarray(src), np.asarray(dst))
    return out
